# revision 26
# baseline (speedup 1.0000x reference)
"""AnomalyAttention Trainium2 kernel (8 NeuronCores, SPMD, no collectives).

Problem: x:(4,2048,512) f32.  Per batch b:
  Q=x@Wq, K=x@Wk, V=x@Wv, sigma=x@Ws
  P[i,j] = gauss(|i-j|, sigma_i) row-normalized   (B,N,N)
  Z = softmax(Q K^T / sqrt(dm)) @ V               (B,N,dm)
Returns (Z, P).

Sharding: core c handles batch b=c//2, query-row half r0=(c%2)*1024.
Each core computes its own K/V (full batch rows) — no cross-core traffic.
The host passes x[b]^T with the core's query rows permuted to columns
0:1023 (key-order permutation leaves softmax@V invariant), so the SPMD
graph uses only compile-time-constant slices.

Score trick: softmax(QK^T) == softmax(x G x^T) with G = Wk @ Wq^T
precomputed on host, so only one score-side projection U = xG is needed.

Engine split: TensorE does U/V projections and scores/Z in bf16
(host-pre-rounded operands, f32 accumulate); sigma is an exact-f32
matmul (n=1, cheap).  ScalarE computes the Gaussian prior in f32 as
Square + Exp(scale=-0.5/sigma^2 per partition) with accum_out row sums;
VectorE evicts PSUM and normalizes.
"""

import sys
from contextlib import ExitStack

for _p in ("/opt/trn_rl_repo", "/opt/pypackages"):
    if _p not in sys.path:
        sys.path.append(_p)

import numpy as np
import ml_dtypes

import concourse.bacc as bacc
import concourse.mybir as mybir
import concourse.tile as tile
from concourse.bass_utils import run_bass_kernel_spmd

DEBUG_PROBES = False

F32 = mybir.dt.float32
BF16 = mybir.dt.bfloat16

B, N, D, DM = 4, 2048, 512, 512
BLK = N // 2            # query rows per core
P = 128                 # partitions
KT_TILES = D // P       # 4 contraction tiles for projections
DM_TILES = DM // P      # 4 dm partition tiles
J_TILES = N // P        # 16 key row tiles
I_TILES = BLK // P      # 8 query row tiles per core
N_CHUNK = 512           # matmul moving free dim
SOFTMAX_SCALE = float(1.0 / np.sqrt(DM))

_CACHED_NC = None


def _build():
    nc = bacc.Bacc("TRN2", target_bir_lowering=False, debug=False, num_devices=8)

    # host-packed inputs (see kernel() for layouts)
    xtbp_ext = nc.declare_dram_parameter("xtbp", [P, KT_TILES * N], BF16,
                                         isOutput=False)
    xsig_ext = nc.declare_dram_parameter("xsig", [P, KT_TILES * BLK], F32,
                                         isOutput=False)
    wbb_ext = nc.declare_dram_parameter("wbb", [P, 2 * KT_TILES * DM], BF16,
                                        isOutput=False)
    wsp_ext = nc.declare_dram_parameter("wsp", [P, KT_TILES], F32,
                                        isOutput=False)
    mrow_ext = nc.declare_dram_parameter("mrow", [P, N], F32, isOutput=False)
    z_ext = nc.declare_dram_parameter("Z", [BLK, DM], F32, isOutput=True)
    p_ext = nc.declare_dram_parameter("P", [BLK, N], F32, isOutput=True)

    with tile.TileContext(nc) as tc:
        with ExitStack() as _stack:
            _e = _stack.enter_context
            const = _e(tc.tile_pool(name="const", bufs=1))
            xtb_pool = _e(tc.tile_pool(name="xtb", bufs=1))
            xsig_pool = _e(tc.tile_pool(name="xsig", bufs=1))
            wbb_pool = _e(tc.tile_pool(name="wbb", bufs=1))
            ut_pool = _e(tc.tile_pool(name="ut", bufs=DM_TILES))
            v_pool = _e(tc.tile_pool(name="vsb", bufs=J_TILES))
            stexp_pool = _e(tc.tile_pool(name="stexp", bufs=4))
            wsq_pool = _e(tc.tile_pool(name="wsq", bufs=1))
            pex_pool = _e(tc.tile_pool(name="pex", bufs=2))
            zout_pool = _e(tc.tile_pool(name="zout", bufs=2))
            riv_pool = _e(tc.tile_pool(name="riv", bufs=2))
            # PSUM: 3 (mm ring) + 4 (z) + 1 (sig/sums shared) = 8 banks
            psmm = _e(tc.tile_pool(name="psmm", bufs=3, space="PSUM"))
            zps = _e(tc.tile_pool(name="zps", bufs=4, space="PSUM"))
            sumps = _e(tc.tile_pool(name="sumps", bufs=1, space="PSUM"))

            # ---- input DMAs, projection inputs first ----
            # xtbp is packed column-block-major: block q holds columns
            # [q*512, (q+1)*512) of all 4 k-tiles, so the first UT group
            # can start after 1 block (0.5MB) + gb instead of all of x.
            QB = KT_TILES * N_CHUNK  # 2048 elements per q-block
            xtb_all = xtb_pool.tile([P, KT_TILES * N], BF16, name="xtb_all")
            wbb_sb = wbb_pool.tile([P, 2 * KT_TILES * DM], BF16, name="wbb_sb")
            nc.sync.dma_start(xtb_all[:, 0:QB], xtbp_ext[:, 0:QB])
            nc.sync.dma_start(wbb_sb[:, 0:KT_TILES * DM],
                              wbb_ext[:, 0:KT_TILES * DM])
            for q in range(1, KT_TILES):
                nc.sync.dma_start(xtb_all[:, q * QB:(q + 1) * QB],
                                  xtbp_ext[:, q * QB:(q + 1) * QB])
            nc.sync.dma_start(wbb_sb[:, KT_TILES * DM:],
                              wbb_ext[:, KT_TILES * DM:])
            wsp_sb = const.tile([P, KT_TILES], F32)
            nc.sync.dma_start(wsp_sb[:], wsp_ext[:, :])
            mrow_sb = const.tile([P, N], F32)
            nc.sync.dma_start(mrow_sb[:], mrow_ext[:, :])
            xsig_sb = xsig_pool.tile([P, KT_TILES * BLK], F32, name="xsig_sb")
            for k in range(KT_TILES):
                nc.sync.dma_start(xsig_sb[:, k * BLK:(k + 1) * BLK],
                                  xsig_ext[:, k * BLK:(k + 1) * BLK])

            def xtbs(k, c0, w):
                q, r = divmod(c0, N_CHUNK)
                assert r + w <= N_CHUNK
                base = q * QB + k * N_CHUNK + r
                return xtb_all[:, base:base + w]

            def gb(k):
                return wbb_sb[:, k * DM:(k + 1) * DM]

            def wvb(k):
                return wbb_sb[:, KT_TILES * DM + k * DM:
                              KT_TILES * DM + (k + 1) * DM]

            ones_sb = const.tile([P, 1], BF16)
            nc.vector.memset(ones_sb[:], 1.0)

            # HAM warm-up: ~3us of throwaway matmuls while input DMAs land
            warm_sb = const.tile([P, N_CHUNK], BF16)
            nc.vector.memset(warm_sb[:], 0.5)
            warm_ps = psmm.tile([P, N_CHUNK], F32, tag="mm", name="warmps")
            for _w in range(8):
                nc.tensor.matmul(
                    warm_ps[:],
                    lhsT=warm_sb[:, 0:P],
                    rhs=warm_sb[:],
                    start=True,
                    stop=True,
                )

            # per-i-tile bias columns for the prior Square (value -128*t)
            pbias_sb = const.tile([P, I_TILES], F32)
            for t_i in range(I_TILES):
                nc.vector.memset(pbias_sb[:, t_i:t_i + 1], float(-P * t_i))

            sig_sb = const.tile([P, I_TILES], F32)
            rec_sb = const.tile([P, I_TILES], F32)
            rsq_sb = const.tile([P, I_TILES], F32)
            scol_sb = const.tile([P, I_TILES], F32)
            rsum_sb = const.tile([P, I_TILES], F32)
            privr_sb = const.tile([P, I_TILES], F32)

            # ---- sigma = x_blk @ Ws  (exact f32, n=1 matmuls) ----
            sigps = sumps.tile([P, I_TILES], F32, tag="sump", name="sigps")
            nc.vector.memset(sigps[:], 0.0)
            for k in range(KT_TILES):
                for mi in range(I_TILES):
                    nc.tensor.matmul(
                        sigps[:, mi:mi + 1],
                        lhsT=xsig_sb[:, k * BLK + mi * P:k * BLK + (mi + 1) * P],
                        rhs=wsp_sb[:, k:k + 1],
                        start=False,
                        stop=(k == KT_TILES - 1),
                        skip_group_check=True,
                    )
            nc.vector.tensor_copy(sig_sb[:], sigps[:])
            # scol = -0.5 / sigma^2
            nc.vector.reciprocal(rec_sb[:], sig_sb[:])
            nc.vector.tensor_mul(rsq_sb[:], rec_sb[:], rec_sb[:])
            nc.vector.tensor_scalar_mul(scol_sb[:], rsq_sb[:], -0.5)

            # ---- Gaussian prior (ScalarE f32) ----
            # mrow[p, j] = j - p - r0  (host-baked); tile t adds -128*t.
            for t_i in range(I_TILES):
                wsq = wsq_pool.tile([P, N], F32, tag="wsq", name=f"wsq{t_i}")
                nc.scalar.activation(
                    wsq[:], mrow_sb[:],
                    mybir.ActivationFunctionType.Square,
                    bias=pbias_sb[:, t_i:t_i + 1], scale=1.0,
                )
                pex = pex_pool.tile([P, N], F32, tag="pex", name=f"pex{t_i}")
                nc.scalar.activation(
                    pex[:], wsq[:],
                    mybir.ActivationFunctionType.Exp,
                    bias=0.0, scale=scol_sb[:, t_i:t_i + 1],
                    accum_out=rsum_sb[:, t_i:t_i + 1],
                )
                nc.vector.reciprocal(
                    privr_sb[:, t_i:t_i + 1], rsum_sb[:, t_i:t_i + 1])
                nc.vector.tensor_scalar_mul(
                    pex[:], pex[:], privr_sb[:, t_i:t_i + 1])
                nc.sync.dma_start(p_ext[t_i * P:(t_i + 1) * P, :], pex[:])

            # ---- projections (bf16 in, f32 psum) ----
            # UT[c, j] = G^T @ xT  (U = xG; scores = U x^T)
            ut = [ut_pool.tile([P, N], BF16, tag="ut", name=f"ut{_m}")
                  for _m in range(DM_TILES)]
            for nch in range(N // N_CHUNK):
                for m in range(DM_TILES):
                    pp = psmm.tile([P, N_CHUNK], F32, tag="mm", name="mmps")
                    for k in range(KT_TILES):
                        nc.tensor.matmul(
                            pp[:],
                            lhsT=gb(k)[:, m * P:(m + 1) * P],
                            rhs=xtbs(k, nch * N_CHUNK, N_CHUNK),
                            start=(k == 0),
                            stop=(k == KT_TILES - 1),
                        )
                    nc.vector.tensor_copy(
                        ut[m][:, nch * N_CHUNK:(nch + 1) * N_CHUNK], pp[:])
            # V[j, dm] natural layout (16 j-tiles x 512)
            vt = [v_pool.tile([P, DM], BF16, tag="vsb", name=f"vt{_m}")
                  for _m in range(J_TILES)]
            for mj in range(J_TILES):
                pp = psmm.tile([P, DM], F32, tag="mm", name="mmps")
                for k in range(KT_TILES):
                    nc.tensor.matmul(
                        pp[:],
                        lhsT=xtbs(k, mj * P, P),
                        rhs=wvb(k)[:, :],
                        start=(k == 0),
                        stop=(k == KT_TILES - 1),
                    )
                nc.vector.tensor_copy(vt[mj][:], pp[:])

            # ---- attention: per 512-row query chunk ----
            # software-pipelined by one j-tile: ST(next) issues before Z(cur)
            # so the PE fills the ScalarE exp latency with score matmuls.
            items = [(c, j) for c in range(BLK // N_CHUNK)
                     for j in range(J_TILES)]

            def emit_st(c, j):
                sp = psmm.tile([P, N_CHUNK], F32, tag="mm",
                               name=f"stps{c}_{j}")
                for k in range(DM_TILES):
                    nc.tensor.matmul(
                        sp[:],
                        lhsT=ut[k][:, j * P:(j + 1) * P],
                        rhs=xtbs(k, c * N_CHUNK, N_CHUNK),
                        start=(k == 0),
                        stop=(k == DM_TILES - 1),
                    )
                se = stexp_pool.tile([P, N_CHUNK], BF16, tag="stexp",
                                     name=f"se{c}_{j}")
                nc.scalar.activation(
                    se[:], sp[:],
                    mybir.ActivationFunctionType.Exp,
                    bias=0.0, scale=SOFTMAX_SCALE,
                )
                return se

            se_cur = emit_st(*items[0])
            zp = None
            sump = None
            for idx, (chunk, j) in enumerate(items):
                ibase = chunk * N_CHUNK
                if j == 0:
                    zp = [zps.tile([P, DM], F32, tag="zps",
                                   name=f"zps{chunk}_{_m}") for _m in range(4)]
                    sump = sumps.tile([P, 4], F32, tag="sump",
                                      name=f"sump{chunk}")
                    nc.vector.memset(sump[:], 0.0)
                se_next = (emit_st(*items[idx + 1])
                           if idx + 1 < len(items) else None)
                for m in range(4):
                    nc.tensor.matmul(
                        zp[m][:],
                        lhsT=se_cur[:, m * P:(m + 1) * P],
                        rhs=vt[j][:, :],
                        start=(j == 0),
                        stop=(j == J_TILES - 1),
                    )
                    nc.tensor.matmul(
                        sump[:, m:m + 1],
                        lhsT=se_cur[:, m * P:(m + 1) * P],
                        rhs=ones_sb[:, 0:1],
                        start=False,
                        stop=(j == J_TILES - 1),
                        skip_group_check=True,
                    )
                se_cur = se_next
                if j == J_TILES - 1:
                    riv = riv_pool.tile([P, 4], F32, tag="riv",
                                        name=f"riv{chunk}")
                    nc.vector.reciprocal(riv[:], sump[:])
                    for m in range(4):
                        zt = zout_pool.tile([P, DM], F32, tag="zout",
                                            name=f"zt{chunk}_{m}")
                        if m % 2 == 0:
                            nc.vector.tensor_scalar_mul(zt[:], zp[m][:],
                                                        riv[:, m:m + 1])
                        else:
                            nc.scalar.mul(zt[:], zp[m][:], riv[:, m:m + 1])
                        row0 = ibase + m * P
                        nc.sync.dma_start(z_ext[row0:row0 + P, :], zt[:])

    nc.finalize()
    return nc


def _get_nc():
    global _CACHED_NC
    if _CACHED_NC is None:
        _CACHED_NC = _build()
    return _CACHED_NC


def _pack_ktiles(a, kt):
    """(kt*P, C) -> (P, kt*C) with column block k = rows k*P:(k+1)*P."""
    p = a.shape[0] // kt
    return np.ascontiguousarray(
        a.reshape(kt, p, a.shape[1]).transpose(1, 0, 2).reshape(p, -1))


def kernel(x, Wq, Wk, Wv, Ws):
    x = np.asarray(x, dtype=np.float32)
    Wq = np.asarray(Wq, dtype=np.float32)
    Wk = np.asarray(Wk, dtype=np.float32)
    Wv = np.asarray(Wv, dtype=np.float32)
    Ws = np.asarray(Ws, dtype=np.float32)

    G = (Wk @ Wq.T).astype(np.float32)
    wbb = np.concatenate(
        [_pack_ktiles(G, KT_TILES), _pack_ktiles(Wv, KT_TILES)],
        axis=1).astype(ml_dtypes.bfloat16)
    wsp = np.ascontiguousarray(Ws[:, 0].reshape(KT_TILES, P).T)

    core_ids = list(range(8))
    in_maps = []
    jj = np.arange(N, dtype=np.float32)[None, :]
    pp = np.arange(P, dtype=np.float32)[:, None]
    for c in core_ids:
        b, half = c // 2, c % 2
        r0 = half * BLK
        xT = x[b].T  # (D, N)
        if half == 1:
            # query rows first; key-order permutation is softmax@V-invariant
            xT = np.concatenate([xT[:, BLK:], xT[:, :BLK]], axis=1)
        xtp = _pack_ktiles(xT, KT_TILES)  # (P, k*N + j)
        # block-major: [q-block][k] with 512-column blocks (see _build)
        xtbp = np.concatenate(
            [xtp[:, k * N + q * N_CHUNK:k * N + (q + 1) * N_CHUNK]
             for q in range(KT_TILES) for k in range(KT_TILES)],
            axis=1).astype(ml_dtypes.bfloat16)
        xsig = _pack_ktiles(xT[:, :BLK], KT_TILES)
        mrow = np.ascontiguousarray(jj - pp - np.float32(r0))
        in_maps.append({
            "xtbp": xtbp, "xsig": xsig, "wbb": wbb, "wsp": wsp, "mrow": mrow,
        })

    nc = _get_nc()
    res = run_bass_kernel_spmd(nc, in_maps, core_ids)

    Z = np.empty((B, N, DM), dtype=np.float32)
    Pout = np.empty((B, N, N), dtype=np.float32)
    for c in core_ids:
        b, half = c // 2, c % 2
        r0 = half * BLK
        Z[b, r0:r0 + BLK, :] = res.results[c]["Z"]
        Pout[b, r0:r0 + BLK, :] = res.results[c]["P"]
    return Z, Pout


# revision 27
# speedup vs baseline: 1.1863x; 1.1863x over previous
"""AnomalyAttention Trainium2 kernel (8 NeuronCores, SPMD, no collectives).

Problem: x:(4,2048,512) f32.  Per batch b:
  Q=x@Wq, K=x@Wk, V=x@Wv, sigma=x@Ws
  P[i,j] = gauss(|i-j|, sigma_i) row-normalized   (B,N,N)
  Z = softmax(Q K^T / sqrt(dm)) @ V               (B,N,dm)
Returns (Z, P).

Sharding: core c handles batch b=c//2, query-row half r0=(c%2)*1024.
Each core computes its own K/V (full batch rows) — no cross-core traffic.
The host passes x[b]^T with the core's query rows permuted to columns
0:1023 (key-order permutation leaves softmax@V invariant), so the SPMD
graph uses only compile-time-constant slices.

Score trick: softmax(QK^T) == softmax(x G x^T) with G = Wk @ Wq^T
precomputed on host, so only one score-side projection U = xG is needed.

Engine split: TensorE does U/V projections and scores/Z in bf16
(host-pre-rounded operands, f32 accumulate); sigma is an exact-f32
matmul (n=1, cheap).  ScalarE computes the Gaussian prior in f32 as
Square + Exp(scale=-0.5/sigma^2 per partition) with accum_out row sums;
VectorE evicts PSUM and normalizes.
"""

import sys
from contextlib import ExitStack

for _p in ("/opt/trn_rl_repo", "/opt/pypackages"):
    if _p not in sys.path:
        sys.path.append(_p)

import numpy as np
import ml_dtypes

import concourse.bacc as bacc
import concourse.mybir as mybir
import concourse.tile as tile
from concourse.bass_utils import run_bass_kernel_spmd

DEBUG_PROBES = False

F32 = mybir.dt.float32
BF16 = mybir.dt.bfloat16

B, N, D, DM = 4, 2048, 512, 512
BLK = N // 2            # query rows per core
P = 128                 # partitions
KT_TILES = D // P       # 4 contraction tiles for projections
DM_TILES = DM // P      # 4 dm partition tiles
J_TILES = N // P        # 16 key row tiles
I_TILES = BLK // P      # 8 query row tiles per core
N_CHUNK = 512           # matmul moving free dim
SOFTMAX_SCALE = float(1.0 / np.sqrt(DM))

_CACHED_NC = None


def _build():
    nc = bacc.Bacc("TRN2", target_bir_lowering=False, debug=False, num_devices=8)

    # host-packed inputs (see kernel() for layouts)
    xtbp_ext = nc.declare_dram_parameter("xtbp", [P, KT_TILES * N], BF16,
                                         isOutput=False)
    xsig_ext = nc.declare_dram_parameter("xsig", [P, KT_TILES * BLK], F32,
                                         isOutput=False)
    wbb_ext = nc.declare_dram_parameter("wbb", [P, 2 * KT_TILES * DM], BF16,
                                        isOutput=False)
    wsp_ext = nc.declare_dram_parameter("wsp", [P, KT_TILES], F32,
                                        isOutput=False)
    mrow_ext = nc.declare_dram_parameter("mrow", [P, N], F32, isOutput=False)
    z_ext = nc.declare_dram_parameter("Z", [BLK, DM], F32, isOutput=True)
    p_ext = nc.declare_dram_parameter("P", [BLK, N], F32, isOutput=True)

    with tile.TileContext(nc) as tc:
        with ExitStack() as _stack:
            _e = _stack.enter_context
            const = _e(tc.tile_pool(name="const", bufs=1))
            xtb_pool = _e(tc.tile_pool(name="xtb", bufs=1))
            xsig_pool = _e(tc.tile_pool(name="xsig", bufs=1))
            wbb_pool = _e(tc.tile_pool(name="wbb", bufs=1))
            ut_pool = _e(tc.tile_pool(name="ut", bufs=DM_TILES))
            v_pool = _e(tc.tile_pool(name="vsb", bufs=J_TILES))
            stexp_pool = _e(tc.tile_pool(name="stexp", bufs=3))
            wsq_pool = _e(tc.tile_pool(name="wsq", bufs=1))
            pex_pool = _e(tc.tile_pool(name="pex", bufs=2))
            zout_pool = _e(tc.tile_pool(name="zout", bufs=2))
            riv_pool = _e(tc.tile_pool(name="riv", bufs=2))
            # PSUM: 3 (mm ring) + 4 (z) + 1 (sig/sums shared) = 8 banks
            psmm = _e(tc.tile_pool(name="psmm", bufs=3, space="PSUM"))
            zps = _e(tc.tile_pool(name="zps", bufs=4, space="PSUM"))
            sumps = _e(tc.tile_pool(name="sumps", bufs=1, space="PSUM"))

            # ---- input DMAs, projection inputs first ----
            # xtbp is packed column-block-major: block q holds columns
            # [q*512, (q+1)*512) of all 4 k-tiles, so the first UT group
            # can start after 1 block (0.5MB) + gb instead of all of x.
            QB = KT_TILES * N_CHUNK  # 2048 elements per q-block
            xtb_all = xtb_pool.tile([P, KT_TILES * N], BF16, name="xtb_all")
            wbb_sb = wbb_pool.tile([P, 2 * KT_TILES * DM], BF16, name="wbb_sb")
            nc.sync.dma_start(xtb_all[:, 0:QB], xtbp_ext[:, 0:QB])
            nc.sync.dma_start(wbb_sb[:, 0:KT_TILES * DM],
                              wbb_ext[:, 0:KT_TILES * DM])
            for q in range(1, KT_TILES):
                nc.sync.dma_start(xtb_all[:, q * QB:(q + 1) * QB],
                                  xtbp_ext[:, q * QB:(q + 1) * QB])
            nc.sync.dma_start(wbb_sb[:, KT_TILES * DM:],
                              wbb_ext[:, KT_TILES * DM:])
            wsp_sb = const.tile([P, KT_TILES], F32)
            nc.sync.dma_start(wsp_sb[:], wsp_ext[:, :])
            mrow_sb = const.tile([P, N], F32)
            nc.sync.dma_start(mrow_sb[:], mrow_ext[:, :])
            xsig_sb = xsig_pool.tile([P, KT_TILES * BLK], F32, name="xsig_sb")
            for k in range(KT_TILES):
                nc.sync.dma_start(xsig_sb[:, k * BLK:(k + 1) * BLK],
                                  xsig_ext[:, k * BLK:(k + 1) * BLK])

            def xtbs(k, c0, w):
                q, r = divmod(c0, N_CHUNK)
                assert r + w <= N_CHUNK
                base = q * QB + k * N_CHUNK + r
                return xtb_all[:, base:base + w]

            def gb(k):
                return wbb_sb[:, k * DM:(k + 1) * DM]

            def wvb(k):
                return wbb_sb[:, KT_TILES * DM + k * DM:
                              KT_TILES * DM + (k + 1) * DM]

            ones_sb = const.tile([P, 1], BF16)
            nc.vector.memset(ones_sb[:], 1.0)

            # HAM warm-up: ~3us of throwaway matmuls while input DMAs land
            warm_sb = const.tile([P, N_CHUNK], BF16)
            nc.vector.memset(warm_sb[:], 0.5)
            warm_ps = psmm.tile([P, N_CHUNK], F32, tag="mm", name="warmps")
            for _w in range(15):
                nc.tensor.matmul(
                    warm_ps[:],
                    lhsT=warm_sb[:, 0:P],
                    rhs=warm_sb[:],
                    start=True,
                    stop=True,
                )

            # per-i-tile bias columns for the prior Square (value -128*t)
            pbias_sb = const.tile([P, I_TILES], F32)
            for t_i in range(I_TILES):
                nc.vector.memset(pbias_sb[:, t_i:t_i + 1], float(-P * t_i))

            sig_sb = const.tile([P, I_TILES], F32)
            rec_sb = const.tile([P, I_TILES], F32)
            rsq_sb = const.tile([P, I_TILES], F32)
            scol_sb = const.tile([P, I_TILES], F32)
            rsum_sb = const.tile([P, I_TILES], F32)
            privr_sb = const.tile([P, I_TILES], F32)

            # ---- sigma = x_blk @ Ws  (exact f32, n=1 matmuls) ----
            sigps = sumps.tile([P, I_TILES], F32, tag="sump", name="sigps")
            nc.vector.memset(sigps[:], 0.0)
            for k in range(KT_TILES):
                for mi in range(I_TILES):
                    nc.tensor.matmul(
                        sigps[:, mi:mi + 1],
                        lhsT=xsig_sb[:, k * BLK + mi * P:k * BLK + (mi + 1) * P],
                        rhs=wsp_sb[:, k:k + 1],
                        start=False,
                        stop=(k == KT_TILES - 1),
                        skip_group_check=True,
                    )
            nc.vector.tensor_copy(sig_sb[:], sigps[:])
            # scol = -0.5 / sigma^2
            nc.vector.reciprocal(rec_sb[:], sig_sb[:])
            nc.vector.tensor_mul(rsq_sb[:], rec_sb[:], rec_sb[:])
            nc.vector.tensor_scalar_mul(scol_sb[:], rsq_sb[:], -0.5)

            # ---- Gaussian prior (ScalarE f32) ----
            # mrow[p, j] = j - p - r0  (host-baked); tile t adds -128*t.
            for t_i in range(I_TILES):
                wsq = wsq_pool.tile([P, N], F32, tag="wsq", name=f"wsq{t_i}")
                nc.scalar.activation(
                    wsq[:], mrow_sb[:],
                    mybir.ActivationFunctionType.Square,
                    bias=pbias_sb[:, t_i:t_i + 1], scale=1.0,
                )
                pex = pex_pool.tile([P, N], F32, tag="pex", name=f"pex{t_i}")
                nc.scalar.activation(
                    pex[:], wsq[:],
                    mybir.ActivationFunctionType.Exp,
                    bias=0.0, scale=scol_sb[:, t_i:t_i + 1],
                    accum_out=rsum_sb[:, t_i:t_i + 1],
                )
                nc.vector.reciprocal(
                    privr_sb[:, t_i:t_i + 1], rsum_sb[:, t_i:t_i + 1])
                nc.vector.tensor_scalar_mul(
                    pex[:], pex[:], privr_sb[:, t_i:t_i + 1])
                nc.sync.dma_start(p_ext[t_i * P:(t_i + 1) * P, :], pex[:])

            # ---- projections (bf16 in, f32 psum) ----
            # UT[c, j] = G^T @ xT  (U = xG; scores = U x^T)
            ut = [ut_pool.tile([P, N], BF16, tag="ut", name=f"ut{_m}")
                  for _m in range(DM_TILES)]
            for nch in range(N // N_CHUNK):
                for m in range(DM_TILES):
                    pp = psmm.tile([P, N_CHUNK], F32, tag="mm", name="mmps")
                    for k in range(KT_TILES):
                        nc.tensor.matmul(
                            pp[:],
                            lhsT=gb(k)[:, m * P:(m + 1) * P],
                            rhs=xtbs(k, nch * N_CHUNK, N_CHUNK),
                            start=(k == 0),
                            stop=(k == KT_TILES - 1),
                        )
                    nc.vector.tensor_copy(
                        ut[m][:, nch * N_CHUNK:(nch + 1) * N_CHUNK], pp[:])
            # V[j, dm] natural layout (16 j-tiles x 512)
            vt = [v_pool.tile([P, DM], BF16, tag="vsb", name=f"vt{_m}")
                  for _m in range(J_TILES)]
            for mj in range(J_TILES):
                pp = psmm.tile([P, DM], F32, tag="mm", name="mmps")
                for k in range(KT_TILES):
                    nc.tensor.matmul(
                        pp[:],
                        lhsT=xtbs(k, mj * P, P),
                        rhs=wvb(k)[:, :],
                        start=(k == 0),
                        stop=(k == KT_TILES - 1),
                    )
                nc.vector.tensor_copy(vt[mj][:], pp[:])

            # ---- attention: per 512-row query chunk ----
            # software-pipelined by one j-tile: ST(next) issues before Z(cur)
            # so the PE fills the ScalarE exp latency with score matmuls.
            items = [(c, j) for c in range(BLK // N_CHUNK)
                     for j in range(J_TILES)]

            def emit_st(c, j):
                sp = psmm.tile([P, N_CHUNK], F32, tag="mm",
                               name=f"stps{c}_{j}")
                for k in range(DM_TILES):
                    nc.tensor.matmul(
                        sp[:],
                        lhsT=ut[k][:, j * P:(j + 1) * P],
                        rhs=xtbs(k, c * N_CHUNK, N_CHUNK),
                        start=(k == 0),
                        stop=(k == DM_TILES - 1),
                    )
                se = stexp_pool.tile([P, N_CHUNK], BF16, tag="stexp",
                                     name=f"se{c}_{j}")
                nc.scalar.activation(
                    se[:], sp[:],
                    mybir.ActivationFunctionType.Exp,
                    bias=0.0, scale=SOFTMAX_SCALE,
                )
                return se

            se_cur = emit_st(*items[0])
            zp = None
            sump = None
            for idx, (chunk, j) in enumerate(items):
                ibase = chunk * N_CHUNK
                if j == 0:
                    zp = [zps.tile([P, DM], F32, tag="zps",
                                   name=f"zps{chunk}_{_m}") for _m in range(4)]
                    sump = sumps.tile([P, 4], F32, tag="sump",
                                      name=f"sump{chunk}")
                    nc.vector.memset(sump[:], 0.0)
                se_next = (emit_st(*items[idx + 1])
                           if idx + 1 < len(items) else None)
                for m in range(4):
                    nc.tensor.matmul(
                        zp[m][:],
                        lhsT=se_cur[:, m * P:(m + 1) * P],
                        rhs=vt[j][:, :],
                        start=(j == 0),
                        stop=(j == J_TILES - 1),
                    )
                    nc.tensor.matmul(
                        sump[:, m:m + 1],
                        lhsT=se_cur[:, m * P:(m + 1) * P],
                        rhs=ones_sb[:, 0:1],
                        start=False,
                        stop=(j == J_TILES - 1),
                        skip_group_check=True,
                    )
                se_cur = se_next
                if j == J_TILES - 1:
                    riv = riv_pool.tile([P, 4], F32, tag="riv",
                                        name=f"riv{chunk}")
                    nc.vector.reciprocal(riv[:], sump[:])
                    for m in range(4):
                        zt = zout_pool.tile([P, DM], F32, tag="zout",
                                            name=f"zt{chunk}_{m}")
                        if m % 2 == 0:
                            nc.vector.tensor_scalar_mul(zt[:], zp[m][:],
                                                        riv[:, m:m + 1])
                        else:
                            nc.scalar.mul(zt[:], zp[m][:], riv[:, m:m + 1])
                        row0 = ibase + m * P
                        nc.sync.dma_start(z_ext[row0:row0 + P, :], zt[:])

    nc.finalize()
    return nc


def _get_nc():
    global _CACHED_NC
    if _CACHED_NC is None:
        _CACHED_NC = _build()
    return _CACHED_NC


def _pack_ktiles(a, kt):
    """(kt*P, C) -> (P, kt*C) with column block k = rows k*P:(k+1)*P."""
    p = a.shape[0] // kt
    return np.ascontiguousarray(
        a.reshape(kt, p, a.shape[1]).transpose(1, 0, 2).reshape(p, -1))


def kernel(x, Wq, Wk, Wv, Ws):
    x = np.asarray(x, dtype=np.float32)
    Wq = np.asarray(Wq, dtype=np.float32)
    Wk = np.asarray(Wk, dtype=np.float32)
    Wv = np.asarray(Wv, dtype=np.float32)
    Ws = np.asarray(Ws, dtype=np.float32)

    G = (Wk @ Wq.T).astype(np.float32)
    wbb = np.concatenate(
        [_pack_ktiles(G, KT_TILES), _pack_ktiles(Wv, KT_TILES)],
        axis=1).astype(ml_dtypes.bfloat16)
    wsp = np.ascontiguousarray(Ws[:, 0].reshape(KT_TILES, P).T)

    core_ids = list(range(8))
    in_maps = []
    jj = np.arange(N, dtype=np.float32)[None, :]
    pp = np.arange(P, dtype=np.float32)[:, None]
    for c in core_ids:
        b, half = c // 2, c % 2
        r0 = half * BLK
        xT = x[b].T  # (D, N)
        if half == 1:
            # query rows first; key-order permutation is softmax@V-invariant
            xT = np.concatenate([xT[:, BLK:], xT[:, :BLK]], axis=1)
        xtp = _pack_ktiles(xT, KT_TILES)  # (P, k*N + j)
        # block-major: [q-block][k] with 512-column blocks (see _build)
        xtbp = np.concatenate(
            [xtp[:, k * N + q * N_CHUNK:k * N + (q + 1) * N_CHUNK]
             for q in range(KT_TILES) for k in range(KT_TILES)],
            axis=1).astype(ml_dtypes.bfloat16)
        xsig = _pack_ktiles(xT[:, :BLK], KT_TILES)
        mrow = np.ascontiguousarray(jj - pp - np.float32(r0))
        in_maps.append({
            "xtbp": xtbp, "xsig": xsig, "wbb": wbb, "wsp": wsp, "mrow": mrow,
        })

    nc = _get_nc()
    res = run_bass_kernel_spmd(nc, in_maps, core_ids)

    Z = np.empty((B, N, DM), dtype=np.float32)
    Pout = np.empty((B, N, N), dtype=np.float32)
    for c in core_ids:
        b, half = c // 2, c % 2
        r0 = half * BLK
        Z[b, r0:r0 + BLK, :] = res.results[c]["Z"]
        Pout[b, r0:r0 + BLK, :] = res.results[c]["P"]
    return Z, Pout
